# revision 13
# baseline (speedup 1.0000x reference)
# kernel.py — PixelCNN++ AutoConvDecoder on 8 TRN2 NeuronCores.
# Pure data parallel: 8 images per core.  Self-contained.
import contextlib
import numpy as np

import concourse.bass as bass
import concourse.tile as tile
from concourse import bacc, mybir
from concourse.bass_utils import run_bass_kernel_spmd

F16 = mybir.dt.float16
F32 = mybir.dt.float32
I32 = mybir.dt.int32
AF = mybir.ActivationFunctionType
ALU = mybir.AluOpType

B, H, W, C = 64, 64, 64, 1
F, M, L = 32, 2, 64
NCORES = 8
BPC = B // NCORES          # images per core
NPAIR = BPC // 2           # image pairs per core

# padded geometries: 1 top pad row, 1 left + 1 right pad col
WP_F, NR_F = 66, 65
WP_H, NR_H = 34, 33
FLEN_F = WP_F * NR_F       # 4290
FLEN_H = WP_H * NR_H       # 1122
XROWS = 66                 # xpad: 2 top pad rows + 64
XLEN = XROWS * WP_F        # 4356

INIT_TAPS = [(-2, -1), (-2, 0), (-2, 1), (-1, -1), (-1, 0), (-1, 1), (0, -1)]


# ----------------------------------------------------------------------------
# host-side packing
# ----------------------------------------------------------------------------
class Packer:
    def __init__(self):
        self.wcols, self.woff = [], 0
        self.bcols, self.boff = [], 0

    def add_w(self, lhs, dup_base=64):
        k, m = lhs.shape
        col = np.zeros((128, m), np.float32)
        col[0:k] = lhs
        col[dup_base:dup_base + k] = lhs
        self.wcols.append(col)
        off = self.woff
        self.woff += m
        return (off, k, m)

    def add_w_full(self, lhs):
        k, m = lhs.shape
        col = np.zeros((128, m), np.float32)
        col[0:k] = lhs
        self.wcols.append(col)
        off = self.woff
        self.woff += m
        return (off, k, m)

    def add_b(self, vec):
        col = np.zeros(128, np.float32)
        col[: len(vec)] = vec
        self.bcols.append(col)
        off = self.boff
        self.boff += 1
        return off

    def finish(self):
        return (np.concatenate(self.wcols, axis=1).astype(np.float16),
                np.stack(self.bcols, axis=1).astype(np.float32))


def pm(v):
    return np.concatenate([v, -v, v, -v])


def taps_of(kh, kw):
    return [(di, dj) for di in range(kh) for dj in range(kw)]


def prep_core(params, latent):
    P = params
    pk = Packer()
    lay = {}

    def celu_lhs(w_tap):
        return np.concatenate([w_tap, -w_tap], axis=1)

    def add_b2(vec):  # returns (b, b+1) offsets
        return (pk.add_b(vec), pk.add_b(vec + 1.0))

    v0w = np.asarray(P['v0w'], np.float32)
    h0aw = np.asarray(P['h0aw'], np.float32)
    h0bw = np.asarray(P['h0bw'], np.float32)
    u_l = np.zeros((7, F), np.float32)
    ul_l = np.zeros((7, F), np.float32)
    for t, (dr_, dc_) in enumerate(INIT_TAPS):
        if dr_ in (-2, -1):
            u_l[t] += v0w[dr_ + 2, dc_ + 1, 0]
        if dr_ == -1:
            ul_l[t] += h0aw[0, dc_ + 1, 0]
        if dc_ == -1 and dr_ in (-1, 0):
            ul_l[t] += h0bw[dr_ + 1, 0, 0]
    lay['init_u'] = pk.add_w(np.concatenate([u_l, -u_l], axis=1))
    lay['init_ul'] = pk.add_w(np.concatenate([ul_l, -ul_l], axis=1))
    lay['init_bu'] = add_b2(pm(np.asarray(P['v0b'], np.float32)))
    bul = np.asarray(P['h0ab'], np.float32) + np.asarray(P['h0bb'], np.float32)
    lay['init_bul'] = add_b2(pm(bul))

    RES = [
        ('d0u', 'ds', 'F', []),
        ('d0ul', 'dr', 'F', [('Eu1', 'wa')]),
        ('d1u', 'ds', 'H', []),
        ('d1ul', 'dr', 'H', [('Eu3', 'wa')]),
        ('u1u', 'ds', 'H', [('Eu3', 'wa')]),
        ('u1ul', 'dr', 'H', [('Eu4', 'wa_u'), ('Eul3', 'wa_s')]),
        ('u0u', 'ds', 'F', [('Eu1', 'wa')]),
        ('u0ul', 'dr', 'F', [('Eu6', 'wa_u'), ('Eul1', 'wa_s')]),
    ]
    lay['res'] = {}
    cond = np.asarray(latent, np.float32)
    for name, kind, resn, auxspec in RES:
        p = {k: np.asarray(v, np.float32) for k, v in P[name].items()}
        kh, kw = (2, 3) if kind == 'ds' else (2, 2)
        e = {'kind': kind, 'res': resn}
        e['c1'] = [pk.add_w(celu_lhs(p['w1'][di, dj])) for (di, dj) in taps_of(kh, kw)]
        b1 = p['b1'] - p['w1'].sum(axis=(0, 1, 2))
        e['aux'] = []
        wa = p.get('wa')
        for srcE, part in auxspec:
            if part == 'wa':
                wpart = wa
            elif part == 'wa_u':
                wpart = np.concatenate([wa[0:32], wa[64:96]], axis=0)
            else:
                wpart = np.concatenate([wa[32:64], wa[96:128]], axis=0)
            e['aux'].append((srcE, pk.add_w(celu_lhs(wpart))))
            b1 = b1 - wpart.sum(axis=0)
        if auxspec:
            b1 = b1 + p['ba']
        e['bMn'] = pk.add_b(pm(b1))
        e['bE'] = pk.add_b(pm(b1) + 1.0)
        e['c2'] = []
        for (di, dj) in taps_of(kh, kw):
            wt = p['w2'][di, dj]
            e['c2'].append(pk.add_w(np.concatenate([wt[:, F:], wt[:, :F]], axis=1)))
        cvec = cond @ p['wc'] + p['b2'] - p['w2'].sum(axis=(0, 1, 2))
        e['sbias'], e['tbias'] = [], []
        for pr in range(NPAIR):
            a_, b_ = 2 * pr, 2 * pr + 1
            sb = np.concatenate([cvec[a_, F:], cvec[a_, :F], cvec[b_, F:], cvec[b_, :F]])
            tb = np.concatenate([cvec[a_, :F], np.zeros(F, np.float32), cvec[b_, :F]])
            e['sbias'].append(pk.add_b(-sb))
            e['tbias'].append(pk.add_b(tb))
        lay['res'][name] = e

    dsu_w = np.asarray(P['dsu_w'], np.float32)
    dsul_w = np.asarray(P['dsul_w'], np.float32)
    lay['dsu'] = [pk.add_w(np.concatenate([dsu_w[di, dj], -dsu_w[di, dj]], axis=1))
                  for (di, dj) in taps_of(2, 3)]
    lay['dsul'] = [pk.add_w(np.concatenate([dsul_w[di, dj], -dsul_w[di, dj]], axis=1))
                   for (di, dj) in taps_of(2, 2)]
    lay['dsu_b'] = add_b2(pm(np.asarray(P['dsu_b'], np.float32)))
    lay['dsul_b'] = add_b2(pm(np.asarray(P['dsul_b'], np.float32)))

    usu_w = np.asarray(P['usu_w'], np.float32)
    usul_w = np.asarray(P['usul_w'], np.float32)
    wf = lambda dy, dx: usu_w[1 - dy, 2 - dx]
    wfl = lambda dy, dx: usul_w[1 - dy, 1 - dx]
    lay['usu'], lay['usul'] = {}, {}
    for dy in range(2):
        lay['usu'][(dy, 0)] = [(0, pk.add_w(celu_lhs(wf(dy, 1))[:, :64]))]
        lay['usu'][(dy, 1)] = [(1, pk.add_w(celu_lhs(wf(dy, 0))[:, :64])),
                               (0, pk.add_w(celu_lhs(wf(dy, 2))[:, :64]))]
        lay['usul'][(dy, 0)] = [(0, pk.add_w(celu_lhs(wfl(dy, 0))[:, :64]))]
        lay['usul'][(dy, 1)] = [(0, pk.add_w(celu_lhs(wfl(dy, 1))[:, :64]))]
    lay['usu_b'] = add_b2(pm(np.asarray(P['usu_b'], np.float32)))
    lay['usul_b'] = add_b2(pm(np.asarray(P['usul_b'], np.float32)))

    outw = np.asarray(P['outw'], np.float32)[0, 0]
    outb = np.asarray(P['outb'], np.float32) - outw.sum(axis=0)
    order = [1, 4, 2, 5, 0, 3]  # -> [mu0, mu1, ls0, ls1, lg0, lg1]
    wh = np.zeros((F, 32), np.float32)
    wh[:, 0:6] = outw[:, order]
    lay['head'] = pk.add_w(wh)
    bh = np.zeros(128, np.float32)
    bh[0:6] = outb[order]
    bh[32:38] = outb[order]
    lay['head_b'] = pk.add_b(bh)

    perm = np.zeros((96, 128), np.float32)
    for j in range(32):
        perm[j, j] = 1.0
        perm[j, 32 + j] = -1.0
        perm[64 + j, 64 + j] = 1.0
        perm[64 + j, 96 + j] = -1.0
    lay['perm'] = pk.add_w_full(perm)
    lay['ident'] = pk.add_w_full(np.eye(128, dtype=np.float32))
    lay['zero_b'] = pk.add_b(np.zeros(128, np.float32))
    lay['one_b'] = pk.add_b(np.ones(128, np.float32))

    wpack, bpack = pk.finish()
    return lay, wpack, bpack


# ----------------------------------------------------------------------------
# device builder
# ----------------------------------------------------------------------------
def interior(ap_tile, wp, r0, nr, c0, ncol, rstep=1, cstep=1):
    r = ap_tile.rearrange("p (r c) -> p r c", c=wp)
    re_ = r0 + (nr - 1) * rstep + 1
    ce_ = c0 + (ncol - 1) * cstep + 1
    return r[:, r0:re_:rstep, c0:ce_:cstep]


class Ctx:
    pass


def build_nc(lay, woff, boff, n_pairs=NPAIR, debug_names=()):
    nc = bacc.Bacc("TRN2", target_bir_lowering=False, debug=False)
    wpack_d = nc.dram_tensor("wpack", [128, woff], F16, kind="ExternalInput").ap()
    bpack_d = nc.dram_tensor("bpack", [128, boff], F32, kind="ExternalInput").ap()
    timg_d = nc.dram_tensor("timg", [BPC, H * W], I32, kind="ExternalInput").ap()
    out_d = nc.dram_tensor("out_partials", [128, BPC], F32, kind="ExternalOutput").ap()
    dbg = {}
    for nm, shp in debug_names:
        dbg[nm] = nc.dram_tensor(nm, shp, F32, kind="ExternalOutput").ap()

    with tile.TileContext(nc) as tc:
        with nc.allow_low_precision(reason="fp16 activations by design"):
            with contextlib.ExitStack() as stk:
                _build_body(stk, tc, lay, wpack_d, bpack_d, timg_d, out_d, dbg, n_pairs)
    nc.compile()
    return nc


def _pad_memset(nc, t, wp, nrows, val):
    nc.gpsimd.memset(t[:, 0:wp], val)
    r = t.rearrange("p (r c) -> p r c", c=wp)
    nc.gpsimd.memset(r[:, 1:nrows, 0:1], val)
    nc.gpsimd.memset(r[:, 1:nrows, wp - 1:wp], val)


def _build_body(stk, tc, lay, wpack_d, bpack_d, timg_d, out_d, dbg, n_pairs):
    nc = tc.nc
    ctx = Ctx()
    ctx.nc, ctx.tc, ctx.lay = nc, tc, lay
    ctx.dbg = dbg

    consts = stk.enter_context(tc.tile_pool(name="consts", bufs=1))
    wp_t = consts.tile([128, wpack_d.shape[1]], F16, tag="wpack")
    nc.sync.dma_start(wp_t[:], wpack_d[:])
    bp_t = consts.tile([128, bpack_d.shape[1]], F32, tag="bpack")
    nc.sync.dma_start(bp_t[:], bpack_d[:])
    ctx.wp, ctx.bp = wp_t, bp_t
    ctx.timg_d = timg_d

    def WA(spec):
        off, k, m = spec
        return wp_t[0:k, off:off + m]

    def WB(spec):
        off, k, m = spec
        return wp_t[64:64 + k, off:off + m]

    def BV(off, lo=0, hi=128):
        return bp_t[lo:hi, off:off + 1]

    ctx.WA, ctx.WB, ctx.BV = WA, WB, BV

    p = {}
    p['ef'] = stk.enter_context(tc.tile_pool(name="ef", bufs=1))
    p['xf'] = stk.enter_context(tc.tile_pool(name="xf", bufs=1))
    p['eh'] = stk.enter_context(tc.tile_pool(name="eh", bufs=1))
    p['xh'] = stk.enter_context(tc.tile_pool(name="xh", bufs=1))
    p['e2'] = stk.enter_context(tc.tile_pool(name="e2", bufs=2))
    p['stg'] = stk.enter_context(tc.tile_pool(name="stg", bufs=2))
    p['head'] = stk.enter_context(tc.tile_pool(name="headp", bufs=1))
    p['misc'] = stk.enter_context(tc.tile_pool(name="misc", bufs=1))
    p['ps1'] = stk.enter_context(tc.tile_pool(name="ps1", bufs=2, space="PSUM"))
    p['ps2'] = stk.enter_context(tc.tile_pool(name="ps2", bufs=1, space="PSUM"))
    p['ps3'] = stk.enter_context(tc.tile_pool(name="ps3", bufs=1, space="PSUM"))
    ctx.pools = p

    # xpad: convert all 8 images once (int32 -> fp16 padded rows)
    xpad = p['misc'].tile([BPC, XLEN], F16, tag="xpad")
    nc.gpsimd.memset(xpad[:], 0.0)
    with tc.tile_pool(name="xintp", bufs=1) as xip:
        xint = xip.tile([BPC, H * W], I32, tag="xint")
        nc.sync.dma_start(xint[:], timg_d[:])
        xpr = xpad.rearrange("p (r c) -> p r c", c=WP_F)
        nc.vector.tensor_scalar_add(
            xpr[:, 2:2 + H, 1:1 + W],
            xint.rearrange("p (r c) -> p r c", c=W)[:], 0.0)

    out_t = p['misc'].tile([128, BPC], F32, tag="outp")
    nc.gpsimd.memset(out_t[:], 0.0)

    for pr in range(n_pairs):
        _build_pair(ctx, pr, xpad, out_t)

    nc.sync.dma_start(out_d[:], out_t[:])


def _mm_quad(ctx, reg, wA, wB, movA, movB, start, stop):
    nc = ctx.nc
    m = wA.shape[-1]
    nc.tensor.matmul(reg[0:m, :], wA, movA, start=start, stop=stop,
                     tile_position=(0, 0), skip_group_check=True)
    nc.tensor.matmul(reg[64:64 + m, :], wB, movB, start=start, stop=stop,
                     tile_position=(64, 64), skip_group_check=True)


def _eprep(ctx, ps, bMn, bE, eap, n, stgpool='stg', suf=''):
    """E = max(ps + bE, exp(min(ps + bMn, 0))) -> eap (fp16)."""
    nc = ctx.nc
    mn = ctx.pools[stgpool].tile([128, n], F16, tag="mn" + suf)
    nc.vector.tensor_scalar(mn[:], ps, ctx.BV(bMn), 0.0, ALU.add, ALU.min)
    pt = ctx.pools[stgpool].tile([128, n], F16, tag="pt" + suf)
    nc.scalar.activation(pt[:], mn[:], AF.Exp)
    nc.vector.scalar_tensor_tensor(eap, ps, ctx.BV(bE), pt[:], ALU.add, ALU.max)


def _xevict(ctx, ps, bX, xap):
    ctx.nc.scalar.activation(xap, ps, AF.Identity, bias=ctx.BV(bX))


def _alloc(ctx, pool, flen, wp, nrows, tag, padval):
    t = ctx.pools[pool].tile([128, flen], F16, tag=tag)
    _pad_memset(ctx.nc, t, wp, nrows, padval)
    return t


def _build_pair(ctx, pr, xpad, out_t):
    nc, pools, lay = ctx.nc, ctx.pools, ctx.lay
    WA, WB, BV = ctx.WA, ctx.WB, ctx.BV
    imgA, imgB = 2 * pr, 2 * pr + 1

    # ---------------- initial convs ----------------
    xrep = pools['misc'].tile([128, XLEN], F16, tag="xrep")
    for t, (dr_, dc_) in enumerate(INIT_TAPS):
        d = dr_ * WP_F + dc_
        nc.sync.dma_start(xrep[t:t + 1, -d:XLEN], xpad[imgA:imgA + 1, 0:XLEN + d])
        nc.sync.dma_start(xrep[64 + t:65 + t, -d:XLEN], xpad[imgB:imgB + 1, 0:XLEN + d])

    Eu0 = _alloc(ctx, 'ef', FLEN_F, WP_F, NR_F, "Eu0", 1.0)
    Eul0 = _alloc(ctx, 'ef', FLEN_F, WP_F, NR_F, "Eul0", 1.0)
    Xu0 = _alloc(ctx, 'xf', FLEN_F, WP_F, NR_F, "Xu0", 0.0)
    Xul0 = _alloc(ctx, 'xf', FLEN_F, WP_F, NR_F, "Xul0", 0.0)

    for g in range(4):
        for wsp, (bo, bo1), Xt, Et in [
                (lay['init_u'], lay['init_bu'], Xu0, Eu0),
                (lay['init_ul'], lay['init_bul'], Xul0, Eul0)]:
            ps = pools['ps1'].tile([128, 1024], F32, tag="c1")
            for s in range(2):
                r0 = 16 * g + 8 * s
                movA = interior(xrep[0:7], WP_F, 2 + r0, 8, 1, 64)
                movB = interior(xrep[64:71], WP_F, 2 + r0, 8, 1, 64)
                _mm_quad(ctx, ps[:, 512 * s:512 * (s + 1)], WA(wsp), WB(wsp),
                         movA, movB, True, True)
            _xevict(ctx, ps[:], bo, interior(Xt[:], WP_F, 1 + 16 * g, 16, 1, 64))
            _eprep(ctx, ps[:], bo, bo1, interior(Et[:], WP_F, 1 + 16 * g, 16, 1, 64), 1024)

    # ---------------- resnets / downsample ----------------
    Xu1 = _alloc(ctx, 'xf', FLEN_F, WP_F, NR_F, "Xu1", 0.0)
    Eu1 = _alloc(ctx, 'ef', FLEN_F, WP_F, NR_F, "Eu1", 1.0)
    _resnet(ctx, 'd0u', pr, Eu0, Xu0, {}, Xu1, Eu1, 'F')
    Xul1 = _alloc(ctx, 'xf', FLEN_F, WP_F, NR_F, "Xul1", 0.0)
    Eul1 = _alloc(ctx, 'ef', FLEN_F, WP_F, NR_F, "Eul1", 1.0)
    _resnet(ctx, 'd0ul', pr, Eul0, Xul0, {'Eu1': Eu1}, Xul1, Eul1, 'F')

    Xu2 = _alloc(ctx, 'xh', FLEN_H, WP_H, NR_H, "Xu2", 0.0)
    Eu2 = _alloc(ctx, 'eh', FLEN_H, WP_H, NR_H, "Eu2", 1.0)
    _strided(ctx, 'dsu', Xu1, Xu2, Eu2, taps_of(2, 3), lay['dsu_b'])
    Xul2 = _alloc(ctx, 'xh', FLEN_H, WP_H, NR_H, "Xul2", 0.0)
    Eul2 = _alloc(ctx, 'eh', FLEN_H, WP_H, NR_H, "Eul2", 1.0)
    _strided(ctx, 'dsul', Xul1, Xul2, Eul2, taps_of(2, 2), lay['dsul_b'])

    Xu3 = _alloc(ctx, 'xh', FLEN_H, WP_H, NR_H, "Xu3", 0.0)
    Eu3 = _alloc(ctx, 'eh', FLEN_H, WP_H, NR_H, "Eu3", 1.0)
    _resnet(ctx, 'd1u', pr, Eu2, Xu2, {}, Xu3, Eu3, 'H')
    Xul3 = _alloc(ctx, 'xh', FLEN_H, WP_H, NR_H, "Xul3", 0.0)
    Eul3 = _alloc(ctx, 'eh', FLEN_H, WP_H, NR_H, "Eul3", 1.0)
    _resnet(ctx, 'd1ul', pr, Eul2, Xul2, {'Eu3': Eu3}, Xul3, Eul3, 'H')
    Xu4 = _alloc(ctx, 'xh', FLEN_H, WP_H, NR_H, "Xu2", 0.0)
    Eu4 = _alloc(ctx, 'eh', FLEN_H, WP_H, NR_H, "Eu2", 1.0)
    _resnet(ctx, 'u1u', pr, Eu3, Xu3, {'Eu3': Eu3}, Xu4, Eu4, 'H')
    Xul4 = _alloc(ctx, 'xh', FLEN_H, WP_H, NR_H, "Xul2", 0.0)
    _resnet(ctx, 'u1ul', pr, Eul3, Xul3, {'Eu4': Eu4, 'Eul3': Eul3}, Xul4, None, 'H')

    Xu5 = _alloc(ctx, 'xf', FLEN_F, WP_F, NR_F, "Xu0", 0.0)
    Eu5 = _alloc(ctx, 'ef', FLEN_F, WP_F, NR_F, "Eu0", 1.0)
    _deconv(ctx, 'usu', Xu4, Xu5, Eu5, lay['usu_b'])
    Xul5 = _alloc(ctx, 'xf', FLEN_F, WP_F, NR_F, "Xul0", 0.0)
    Eul5 = _alloc(ctx, 'ef', FLEN_F, WP_F, NR_F, "Eul0", 1.0)
    _deconv(ctx, 'usul', Xul4, Xul5, Eul5, lay['usul_b'])

    Eu6 = _alloc(ctx, 'ef', FLEN_F, WP_F, NR_F, "Eu0", 1.0)
    _resnet(ctx, 'u0u', pr, Eu5, Xu5, {'Eu1': Eu1}, None, Eu6, 'F')
    Eul6 = _alloc(ctx, 'ef', FLEN_F, WP_F, NR_F, "Eul0", 1.0)
    _resnet(ctx, 'u0ul', pr, Eul5, Xul5, {'Eu6': Eu6, 'Eul1': Eul1}, None, Eul6, 'F')

    _head(ctx, pr, Eul6, out_t)


def _resnet(ctx, name, pr, Ein, Xin, auxE, Xout, Eout, res):
    nc, pools, lay = ctx.nc, ctx.pools, ctx.lay
    WA, WB, BV = ctx.WA, ctx.WB, ctx.BV
    e = lay['res'][name]
    kh, kw = (2, 3) if e['kind'] == 'ds' else (2, 2)
    wp = WP_F if res == 'F' else WP_H
    flen = FLEN_F if res == 'F' else FLEN_H
    nrows_t = NR_F if res == 'F' else NR_H
    ngroups = 4 if res == 'F' else 1
    rpc = 8 if res == 'F' else 16     # rows per 512-chunk
    cols = 64 if res == 'F' else 32

    E2 = _alloc(ctx, 'e2', flen, wp, nrows_t, "E2" + res, 1.0)

    for g in range(4 if res == 'F' else 1):
        ps1 = pools['ps1'].tile([128, 1024], F32, tag="c1")
        for s in range(2):
            r0 = rpc * (2 * g + s)
            reg = ps1[:, 512 * s:512 * (s + 1)]
            taps = taps_of(kh, kw)
            for t, (di, dj) in enumerate(taps):
                movA = interior(Ein[0:64], wp, r0 + di, rpc, dj, cols)
                movB = interior(Ein[64:128], wp, r0 + di, rpc, dj, cols)
                _mm_quad(ctx, reg, WA(e['c1'][t]), WB(e['c1'][t]), movA, movB,
                         t == 0, t == len(taps) - 1 and not e['aux'])
            for i, (srcE, wsp) in enumerate(e['aux']):
                At = auxE[srcE]
                movA = interior(At[0:64], wp, 1 + r0, rpc, 1, cols)
                movB = interior(At[64:128], wp, 1 + r0, rpc, 1, cols)
                _mm_quad(ctx, reg, WA(wsp), WB(wsp), movA, movB,
                         False, i == len(e['aux']) - 1)
        eap = interior(E2[:], wp, 1 + rpc * 2 * g, rpc * 2, 1, cols)
        _eprep(ctx, ps1[:], e['bMn'], e['bE'], eap, 1024)

        ps2 = pools['ps2'].tile([128, 1024], F32, tag="c2")
        for s in range(2):
            r0 = rpc * (2 * g + s)
            reg = ps2[:, 512 * s:512 * (s + 1)]
            taps = taps_of(kh, kw)
            for t, (di, dj) in enumerate(taps):
                movA = interior(E2[0:64], wp, r0 + di, rpc, dj, cols)
                movB = interior(E2[64:128], wp, r0 + di, rpc, dj, cols)
                _mm_quad(ctx, reg, WA(e['c2'][t]), WB(e['c2'][t]), movA, movB,
                         t == 0, t == len(taps) - 1)

        q = pools['stg'].tile([128, 1024], F32, tag="q")
        nc.scalar.activation(q[:], ps2[:], AF.Exp, scale=-1.0, bias=BV(e['sbias'][pr]))
        r = pools['stg'].tile([128, 1024], F32, tag="r")
        nc.vector.tensor_scalar_add(r[:], q[:], 1.0)
        S = pools['stg'].tile([128, 1024], F16, tag="s")
        nc.vector.reciprocal(S[:], r[:])
        T = pools['stg'].tile([96, 1024], F16, tag="t")
        nc.gpsimd.memset(T[32:64, :], 0.0)
        nc.vector.scalar_tensor_tensor(T[0:32, :], ps2[32:64, :], BV(e['tbias'][pr], 0, 32),
                                       S[0:32, :], ALU.add, ALU.mult)
        nc.vector.scalar_tensor_tensor(T[64:96, :], ps2[96:128, :], BV(e['tbias'][pr], 64, 96),
                                       S[64:96, :], ALU.add, ALU.mult)
        ps3 = pools['ps3'].tile([128, 1024], F32, tag="t3")
        po = lay['perm'][0]
        io_ = lay['ident'][0]
        for s in range(2):
            reg = ps3[:, 512 * s:512 * (s + 1)]
            nc.tensor.matmul(reg, ctx.wp[0:96, po:po + 128], T[:, 512 * s:512 * (s + 1)],
                             start=True, stop=False, skip_group_check=True)
            r0 = rpc * (2 * g + s)
            movX = interior(Xin[:], wp, 1 + r0, rpc, 1, cols)
            nc.tensor.matmul(reg, ctx.wp[0:128, io_:io_ + 128], movX,
                             start=False, stop=True, skip_group_check=True)
        if Xout is not None:
            _xevict(ctx, ps3[:], lay['zero_b'],
                    interior(Xout[:], wp, 1 + rpc * 2 * g, rpc * 2, 1, cols))
        if Eout is not None:
            eap = interior(Eout[:], wp, 1 + rpc * 2 * g, rpc * 2, 1, cols)
            _eprep(ctx, ps3[:], lay['zero_b'], lay['one_b'], eap, 1024)


def _strided(ctx, key, Xin, Xout, Eout, taps, bpair):
    nc, pools, lay = ctx.nc, ctx.pools, ctx.lay
    specs = lay[key]
    ps = pools['ps1'].tile([128, 1024], F32, tag="c1")
    for s in range(2):
        reg = ps[:, 512 * s:512 * (s + 1)]
        for t, (di, dj) in enumerate(taps):
            movA = interior(Xin[0:32], WP_F, 32 * s + di, 16, dj, 32, rstep=2, cstep=2)
            movB = interior(Xin[64:96], WP_F, 32 * s + di, 16, dj, 32, rstep=2, cstep=2)
            _mm_quad(ctx, reg, ctx.WA(specs[t]), ctx.WB(specs[t]), movA, movB,
                     t == 0, t == len(taps) - 1)
    bo, bo1 = bpair
    _xevict(ctx, ps[:], bo, interior(Xout[:], WP_H, 1, 32, 1, 32))
    _eprep(ctx, ps[:], bo, bo1, interior(Eout[:], WP_H, 1, 32, 1, 32), 1024)


def _deconv(ctx, key, Xin, Xout, Eout, bpair):
    nc, pools, lay = ctx.nc, ctx.pools, ctx.lay
    bo, bo1 = bpair
    for dy in range(2):
        for codd in range(2):
            specs = lay[key][(dy, codd)]
            ps = pools['ps1'].tile([128, 1024], F32, tag="c1")
            for s in range(2):
                reg = ps[:, 512 * s:512 * (s + 1)]
                for t, (dcol, wsp) in enumerate(specs):
                    movA = interior(Xin[0:32], WP_H, 1 + 16 * s, 16, 1 + dcol, 32)
                    movB = interior(Xin[64:96], WP_H, 1 + 16 * s, 16, 1 + dcol, 32)
                    _mm_quad(ctx, reg, ctx.WA(wsp), ctx.WB(wsp), movA, movB,
                             t == 0, t == len(specs) - 1)
            _xevict(ctx, ps[:], bo,
                    interior(Xout[:], WP_F, 1 + dy, 32, 1 + codd, 32, rstep=2, cstep=2))
            _eprep(ctx, ps[:], bo, bo1,
                   interior(Eout[:], WP_F, 1 + dy, 32, 1 + codd, 32, rstep=2, cstep=2),
                   1024)


def _head(ctx, pr, Eul6, out_t):
    nc, pools, lay = ctx.nc, ctx.pools, ctx.lay
    BV = ctx.BV
    imgA, imgB = 2 * pr, 2 * pr + 1
    ho = lay['head'][0]

    stg = pools['head'].tile([64, 4096], F16, tag="hstg")
    for g in range(4):
        ps = pools['ps2'].tile([64, 1024], F32, tag="c2")
        for s in range(2):
            c = 2 * g + s
            reg = ps[:, 512 * s:512 * (s + 1)]
            movA = interior(Eul6[0:32], WP_F, 1 + 8 * c, 8, 1, 64)
            movB = interior(Eul6[64:96], WP_F, 1 + 8 * c, 8, 1, 64)
            nc.tensor.matmul(reg[0:32, :], ctx.wp[0:32, ho:ho + 32], movA,
                             start=True, stop=True, tile_position=(0, 0),
                             skip_group_check=True)
            nc.tensor.matmul(reg[32:64, :], ctx.wp[64:96, ho:ho + 32], movB,
                             start=True, stop=True, tile_position=(64, 32),
                             skip_group_check=True)
        nc.scalar.activation(stg[:, 1024 * g:1024 * (g + 1)], ps[:], AF.Identity,
                             bias=BV(lay['head_b'], 0, 64))

    pxstg = pools['head'].tile([128, 1024], F16, tag="pxstg")
    for k in range(32):
        nc.sync.dma_start(pxstg[:, 32 * k:32 * k + 16],
                          stg[0:16, 128 * k:128 * (k + 1)], transpose=True)
        nc.sync.dma_start(pxstg[:, 32 * k + 16:32 * k + 32],
                          stg[32:48, 128 * k:128 * (k + 1)], transpose=True)

    p4 = pxstg.rearrange("p (k i c) -> p k i c", i=2, c=16)
    mu = p4[:, :, :, 0:2]
    ls = p4[:, :, :, 2:4]
    lg = p4[:, :, :, 4:6]

    ixv = pools['head'].tile([128, 64], I32, tag="ixv")
    for img, gidx in ((imgA, 0), (imgB, 1)):
        src = ctx.timg_d[img:img + 1, :].rearrange("o (k p) -> o p k", p=128)
        nc.sync.dma_start(
            ixv.rearrange("p (k i) -> p k i", i=2)[:, :, gidx:gidx + 1], src)
    xv = pools['head'].tile([128, 64], F32, tag="xv")
    nc.vector.tensor_scalar(xv[:], ixv[:], 2.0, -1.0, ALU.mult, ALU.add)

    hp = pools['head']
    t1 = hp.tile([128, 128], F32, tag="h1")
    nc.vector.tensor_scalar_max(t1[:], ls, -7.0)
    invs = hp.tile([128, 128], F32, tag="h2")
    nc.scalar.activation(invs[:], t1[:], AF.Exp, scale=-1.0)
    zz = hp.tile([128, 128], F32, tag="h3")
    nc.vector.tensor_tensor(zz[:], invs[:], mu, ALU.mult)
    xvb = xv.rearrange("p (k i) -> p k i", i=2).broadcast_to([128, 32, 2, 2])
    z = hp.tile([128, 128], F32, tag="h4")
    nc.vector.tensor_tensor(z[:], zz[:], xvb, ALU.mult)
    # log_sigmoid(z) applied in product space:
    # lp = m - ln(1+exp(2m-z)), m=min(z,0);  e^lp = e^m / (1+exp(2m-z))
    mt = hp.tile([128, 128], F32, tag="h5")
    nc.vector.tensor_scalar_min(mt[:], z[:], 0.0)
    na = hp.tile([128, 128], F32, tag="h5b")
    nc.vector.scalar_tensor_tensor(na[:], mt[:], 2.0, z[:], ALU.mult, ALU.subtract)
    q2 = hp.tile([128, 128], F32, tag="h5c")
    nc.scalar.activation(q2[:], na[:], AF.Exp)
    em = hp.tile([128, 128], F32, tag="h5d")
    nc.scalar.activation(em[:], mt[:], AF.Exp)
    cmax = hp.tile([128, 64], F32, tag="h6")
    nc.vector.tensor_tensor(cmax[:], p4[:, :, :, 4:5], p4[:, :, :, 5:6], ALU.max)
    cmb = cmax.rearrange("p (k i) -> p k i", i=2).broadcast_to([128, 32, 2, 2])
    dd = hp.tile([128, 128], F32, tag="h7")
    nc.vector.tensor_tensor(dd[:], lg, cmb, ALU.subtract)
    e2 = hp.tile([128, 128], F32, tag="h10")
    nc.scalar.activation(e2[:], dd[:], AF.Exp)
    r2 = hp.tile([128, 128], F32, tag="h8")
    nc.vector.tensor_scalar_add(r2[:], q2[:], 1.0)
    rr = hp.tile([128, 128], F32, tag="h8b")
    nc.vector.reciprocal(rr[:], r2[:])
    elp = hp.tile([128, 128], F32, tag="h9")
    nc.vector.tensor_tensor(elp[:], em[:], rr[:], ALU.mult)
    e1 = hp.tile([128, 128], F32, tag="h9b")
    nc.vector.tensor_tensor(e1[:], e2[:], elp[:], ALU.mult)
    e1r = e1.rearrange("p (q m) -> p q m", m=2)
    n1 = hp.tile([128, 64], F32, tag="h11")
    nc.vector.tensor_tensor(n1[:], e1r[:, :, 0:1], e1r[:, :, 1:2], ALU.add)
    e2r = e2.rearrange("p (q m) -> p q m", m=2)
    n2 = hp.tile([128, 64], F32, tag="h12")
    nc.vector.tensor_tensor(n2[:], e2r[:, :, 0:1], e2r[:, :, 1:2], ALU.add)
    l1 = hp.tile([128, 64], F32, tag="h13")
    nc.scalar.activation(l1[:], n1[:], AF.Ln)
    l2 = hp.tile([128, 64], F32, tag="h14")
    nc.scalar.activation(l2[:], n2[:], AF.Ln)
    mix = hp.tile([128, 64], F32, tag="h15")
    nc.vector.tensor_tensor(mix[:], l1[:], l2[:], ALU.subtract)
    mixr = mix.rearrange("p (k i) -> p k i", i=2)
    nc.vector.tensor_reduce(out_t[:, imgA:imgA + 1], mixr[:, :, 0:1],
                            mybir.AxisListType.XY, ALU.add)
    nc.vector.tensor_reduce(out_t[:, imgB:imgB + 1], mixr[:, :, 1:2],
                            mybir.AxisListType.XY, ALU.add)


# ----------------------------------------------------------------------------
# entry point
# ----------------------------------------------------------------------------
_CACHE = {}
_last_cores = None


def kernel(latent_tensor, params, train_data):
    latent = np.asarray(latent_tensor, np.float32)
    td = np.asarray(train_data)
    cores = []
    lay0 = None
    for c in range(NCORES):
        sl = slice(c * BPC, (c + 1) * BPC)
        lay, wpack, bpack = prep_core(params, latent[sl])
        if lay0 is None:
            lay0 = lay
        cores.append({
            "wpack": wpack,
            "bpack": bpack,
            "timg": td[sl].reshape(BPC, H * W).astype(np.int32),
        })
    global _last_cores
    _last_cores = cores
    key = (cores[0]["wpack"].shape[1], cores[0]["bpack"].shape[1])
    if key not in _CACHE:
        _CACHE[key] = build_nc(lay0, key[0], key[1])
    nc = _CACHE[key]
    res = run_bass_kernel_spmd(nc, cores, core_ids=list(range(NCORES)))
    outs = [res.results[c]["out_partials"].sum(axis=0) for c in range(NCORES)]
    return np.concatenate(outs).astype(np.float32)
